# revision 6
# baseline (speedup 1.0000x reference)
"""Contrastive loss kernel for Trainium2 (8 NeuronCores, data-parallel).

Reference math (per even/odd row pair i):
    x  = query[2i], y1 = embed[2i], y2 = embed[2i+1]
    pos = <x,y1> / (|x||y1|),  neg = <x,y2> / (|x||y2|)
    loss_i = log(1 + exp(neg - pos));  output = mean_i(loss_i)

Strategy: all five length-512 reductions per row (x·y1, x·y2, x·x, y1·y1,
y2·y2) run on the TensorEngine as 32x32 gram matmuls over a d-major
layout (partition = dim%128, contraction over 4 chunk-matmuls accumulated
in PSUM). The needed values are the gram diagonals; with 32-row groups
placed at partition offset 32g, the diagonal of each group lands at
column (p mod 32), so one host-built identity mask + a grouped
tensor_reduce extracts all 5 stats per block in two DVE ops. This moves
the O(B*D) reduction work off DVE/ACT (the baseline bottleneck) onto the
otherwise-idle PE. Inputs stream as bf16 (halves HBM traffic vs f32);
PSUM/stats/epilogue stay f32.
"""

import numpy as np
from contextlib import ExitStack

import concourse.bass as bass
import concourse.bacc as bacc
import concourse.tile as tile
from concourse import mybir
from concourse.bass_utils import run_bass_kernel_spmd

N_CORES = 8
B = 65536
D = 512
PAIRS = B // 2                       # 32768
ROWS_PER_CORE = PAIRS // N_CORES     # 4096
NBLK = ROWS_PER_CORE // 128          # 32 blocks of 128 rows
SUP = 8                              # blocks per DMA supertile (8 KiB lines)
NSUP = NBLK // SUP
NCH = D // 128                       # 4 contraction chunks
NG = 4                               # 32-row groups per block
NST = 5                              # stats per row

F32 = mybir.dt.float32
BF16 = mybir.dt.bfloat16
A = mybir.ActivationFunctionType
ALU = mybir.AluOpType

DT_IN = BF16


def _body(ctx, tc, out_ap, x_ap, y1_ap, y2_ap, mask_ap):
    nc = tc.nc

    xin = ctx.enter_context(tc.tile_pool(name="xin", bufs=2))
    y1in = ctx.enter_context(tc.tile_pool(name="y1in", bufs=2))
    y2in = ctx.enter_context(tc.tile_pool(name="y2in", bufs=2))
    psp = ctx.enter_context(
        tc.tile_pool(name="psp", bufs=4, space=bass.MemorySpace.PSUM))
    scr = ctx.enter_context(tc.tile_pool(name="scr", bufs=4))
    stats = ctx.enter_context(tc.tile_pool(name="stats", bufs=1))
    epi = ctx.enter_context(tc.tile_pool(name="epi", bufs=1))

    mask = stats.tile([128, NST, 32], F32, tag="mask")
    nc.sync.dma_start(out=mask[:], in_=mask_ap)
    statsT = stats.tile([128, NBLK, NST], F32, tag="statsT")

    for s in range(NSUP):
        lo, hi = s * SUP * D, (s + 1) * SUP * D
        xt = xin.tile([128, SUP * D], DT_IN, tag="xt", name="xt")
        nc.sync.dma_start(out=xt[:], in_=x_ap[:, lo:hi])
        y1t = y1in.tile([128, SUP * D], DT_IN, tag="y1t", name="y1t")
        nc.sync.dma_start(out=y1t[:], in_=y1_ap[:, lo:hi])
        y2t = y2in.tile([128, SUP * D], DT_IN, tag="y2t", name="y2t")
        nc.sync.dma_start(out=y2t[:], in_=y2_ap[:, lo:hi])

        for j in range(SUP):
            b = s * SUP + j
            pb = psp.tile([128, NST, 32], F32, tag="pb", name="pb")
            for c in range(NCH):
                base = j * D + c * 128
                for g in range(NG):
                    sl = slice(base + 32 * g, base + 32 * g + 32)
                    xg, y1g, y2g = xt[:, sl], y1t[:, sl], y2t[:, sl]
                    pairs = ((xg, y1g), (xg, y2g), (xg, xg),
                             (y1g, y1g), (y2g, y2g))
                    for st, (vv, ww) in enumerate(pairs):
                        nc.tensor.matmul(
                            out=pb[32 * g:32 * g + 32, st, :],
                            lhsT=vv, rhs=ww,
                            start=(c == 0), stop=(c == NCH - 1),
                            tile_position=(0, 32 * g))
            prod = scr.tile([128, NST, 32], F32, tag="prod", name="prod")
            nc.vector.tensor_tensor(out=prod[:], in0=pb[:], in1=mask[:],
                                    op=ALU.mult)
            nc.vector.tensor_reduce(out=statsT[:, b, :], in_=prod[:],
                                    op=ALU.add, axis=mybir.AxisListType.X)

    # Epilogue on [128, NBLK] strided stat views.
    # rsqrt(q) = Exp(-0.5 * Ln(q)); Exp/Ln share one ACT table set.
    dxy1 = statsT[:, :, 0]
    dxy2 = statsT[:, :, 1]
    sx = statsT[:, :, 2]
    sy1 = statsT[:, :, 3]
    sy2 = statsT[:, :, 4]

    def et(name):
        return epi.tile([128, NBLK], F32, tag=name, name=name)

    q1, q2 = et("q1"), et("q2")
    nc.vector.tensor_mul(q1[:], sx, sy1)
    nc.vector.tensor_mul(q2[:], sx, sy2)
    l1, l2 = et("l1"), et("l2")
    nc.scalar.activation(out=l1[:], in_=q1[:], func=A.Ln)
    nc.scalar.activation(out=l2[:], in_=q2[:], func=A.Ln)
    r1, r2 = et("r1"), et("r2")
    nc.scalar.activation(out=r1[:], in_=l1[:], func=A.Exp, scale=-0.5)
    nc.scalar.activation(out=r2[:], in_=l2[:], func=A.Exp, scale=-0.5)
    pos, neg = et("pos"), et("neg")
    nc.vector.tensor_mul(pos[:], dxy1, r1[:])
    nc.vector.tensor_mul(neg[:], dxy2, r2[:])
    z = et("z")
    nc.vector.tensor_sub(z[:], neg[:], pos[:])
    e = et("e")
    nc.scalar.activation(out=e[:], in_=z[:], func=A.Exp)
    loss = et("loss")
    nc.scalar.activation(out=loss[:], in_=e[:], func=A.Ln, bias=1.0)
    nc.sync.dma_start(out=out_ap, in_=loss[:])


def _build():
    nc = bacc.Bacc("TRN2", target_bir_lowering=False, debug=False,
                   num_devices=N_CORES)
    x = nc.dram_tensor("x", [128, NBLK * D], DT_IN, kind="ExternalInput").ap()
    y1 = nc.dram_tensor("y1", [128, NBLK * D], DT_IN,
                        kind="ExternalInput").ap()
    y2 = nc.dram_tensor("y2", [128, NBLK * D], DT_IN,
                        kind="ExternalInput").ap()
    mask = nc.dram_tensor("mask", [128, NST * 32], F32,
                          kind="ExternalInput").ap()
    out = nc.dram_tensor("out", [128, NBLK], F32, kind="ExternalOutput").ap()
    with tile.TileContext(nc) as tc:
        with ExitStack() as ctx:
            _body(ctx, tc, out[:], x[:], y1[:], y2[:], mask[:])
    nc.compile()
    return nc


_NC_CACHE = None


def _get_nc():
    global _NC_CACHE
    if _NC_CACHE is None:
        _NC_CACHE = _build()
    return _NC_CACHE


def _layout(a_rows):
    # [4096, 512] -> d-major [128, NBLK*512]: partition p holds dim
    # c*128+p of every row; free axis is [blk, chunk, row-in-block].
    import ml_dtypes
    a = a_rows.reshape(NBLK, 128, NCH, 128).transpose(3, 0, 2, 1)
    return np.ascontiguousarray(a.reshape(128, NBLK * D)
                                .astype(ml_dtypes.bfloat16))


def _mask_np():
    # mask[p, st*32 + j] = 1.0 where j == p % 32
    m = np.zeros((128, NST, 32), dtype=np.float32)
    p = np.arange(128)
    for st in range(NST):
        m[p, st, p % 32] = 1.0
    return m.reshape(128, NST * 32)


def _in_maps(query, embed):
    x1 = query[0::2]
    e1 = embed[0::2]
    e2 = embed[1::2]
    mask = _mask_np()
    maps = []
    for c in range(N_CORES):
        sl = slice(c * ROWS_PER_CORE, (c + 1) * ROWS_PER_CORE)
        maps.append({"x": _layout(x1[sl]), "y1": _layout(e1[sl]),
                     "y2": _layout(e2[sl]), "mask": mask})
    return maps


def kernel(query, embed, y, _trace=False):
    query = np.asarray(query, dtype=np.float32)
    embed = np.asarray(embed, dtype=np.float32)
    nc = _get_nc()
    res = run_bass_kernel_spmd(nc, _in_maps(query, embed),
                               core_ids=list(range(N_CORES)), trace=_trace)
    total = 0.0
    for c in range(N_CORES):
        total += res.results[c]["out"].astype(np.float64).sum()
    if _trace:
        kernel._last_results = res
    return np.float32(total / PAIRS)


# revision 7
# speedup vs baseline: 1.3802x; 1.3802x over previous
"""Contrastive loss kernel for Trainium2 (8 NeuronCores, data-parallel).

Reference math (per even/odd row pair i):
    x  = query[2i], y1 = embed[2i], y2 = embed[2i+1]
    pos = <x,y1> / (|x||y1|),  neg = <x,y2> / (|x||y2|)
    loss_i = log(1 + exp(neg - pos));  output = mean_i(loss_i)

query[1::2] and y are unused by the math. Each core processes 4096 pairs
as 32 blocks of 128 rows (row-major, partition = row). Five fused
length-512 reductions per block run on DVE (scalar_tensor_tensor) and ACT
(Square+accum); these ops run at a fixed elems/cycle independent of dtype
(no DVE perf modes on accumulating ops; PE grams lose to a ~180 ns fixed
per-matmul cost), so the win over the f32 baseline is bf16 input
streaming — half the HBM traffic — plus a measured-cost engine balance:
DVE 660 ns/op gets the 2 dots + x**2 on 6 of 8 blocks, ACT 825 ns/op
gets the rest (22 vs 18 ops per 8 blocks). Stats/epilogue stay f32.
"""

import numpy as np
from contextlib import ExitStack

import concourse.bass as bass
import concourse.bacc as bacc
import concourse.tile as tile
from concourse import mybir
from concourse.bass_utils import run_bass_kernel_spmd

N_CORES = 8
B = 65536
D = 512
PAIRS = B // 2                       # 32768
ROWS_PER_CORE = PAIRS // N_CORES     # 4096
NBLK = ROWS_PER_CORE // 128          # 32 blocks of 128 rows
SUP = 8                              # blocks per DMA supertile (8 KiB lines)
NSUP = NBLK // SUP

F32 = mybir.dt.float32
BF16 = mybir.dt.bfloat16
A = mybir.ActivationFunctionType
ALU = mybir.AluOpType

DT_IN = BF16


def _body(ctx, tc, out_ap, x_ap, y1_ap, y2_ap):
    nc = tc.nc

    xin = ctx.enter_context(tc.tile_pool(name="xin", bufs=2))
    y1in = ctx.enter_context(tc.tile_pool(name="y1in", bufs=2))
    y2in = ctx.enter_context(tc.tile_pool(name="y2in", bufs=2))
    scrv = ctx.enter_context(tc.tile_pool(name="scrv", bufs=4))
    scra = ctx.enter_context(tc.tile_pool(name="scra", bufs=4))
    stats = ctx.enter_context(tc.tile_pool(name="stats", bufs=1))
    epi = ctx.enter_context(tc.tile_pool(name="epi", bufs=1))

    dxy1 = stats.tile([128, NBLK], F32, tag="dxy1")
    dxy2 = stats.tile([128, NBLK], F32, tag="dxy2")
    sx = stats.tile([128, NBLK], F32, tag="sx")
    sy1 = stats.tile([128, NBLK], F32, tag="sy1")
    sy2 = stats.tile([128, NBLK], F32, tag="sy2")

    def dve_red(in0, in1, acc):
        sv = scrv.tile([128, D], DT_IN, tag="sv", name="sv")
        nc.vector.scalar_tensor_tensor(
            out=sv[:], in0=in0, scalar=1.0, in1=in1,
            op0=ALU.mult, op1=ALU.mult, accum_out=acc)

    def act_sq(in0, acc):
        sa = scra.tile([128, D], DT_IN, tag="sa", name="sa")
        nc.scalar.activation(out=sa[:], in_=in0, func=A.Square, accum_out=acc)

    for s in range(NSUP):
        lo, hi = s * SUP * D, (s + 1) * SUP * D
        xt = xin.tile([128, SUP * D], DT_IN, tag="xt", name="xt")
        nc.sync.dma_start(out=xt[:], in_=x_ap[:, lo:hi])
        y1t = y1in.tile([128, SUP * D], DT_IN, tag="y1t", name="y1t")
        nc.sync.dma_start(out=y1t[:], in_=y1_ap[:, lo:hi])
        y2t = y2in.tile([128, SUP * D], DT_IN, tag="y2t", name="y2t")
        nc.sync.dma_start(out=y2t[:], in_=y2_ap[:, lo:hi])

        for j in range(SUP):
            b = s * SUP + j
            xs = xt[:, j * D:(j + 1) * D]
            y1s = y1t[:, j * D:(j + 1) * D]
            y2s = y2t[:, j * D:(j + 1) * D]

            dve_red(xs, y1s, dxy1[:, b:b + 1])
            dve_red(xs, y2s, dxy2[:, b:b + 1])
            if b % 8 < 6:
                dve_red(xs, xs, sx[:, b:b + 1])
            else:
                act_sq(xs, sx[:, b:b + 1])
            act_sq(y1s, sy1[:, b:b + 1])
            act_sq(y2s, sy2[:, b:b + 1])

    # Epilogue on [128, NBLK] stats.
    # rsqrt(q) = Exp(-0.5 * Ln(q)); Square/Exp/Ln share one ACT table set.
    def et(name):
        return epi.tile([128, NBLK], F32, tag=name, name=name)

    q1, q2 = et("q1"), et("q2")
    nc.vector.tensor_mul(q1[:], sx[:], sy1[:])
    nc.vector.tensor_mul(q2[:], sx[:], sy2[:])
    l1, l2 = et("l1"), et("l2")
    nc.scalar.activation(out=l1[:], in_=q1[:], func=A.Ln)
    nc.scalar.activation(out=l2[:], in_=q2[:], func=A.Ln)
    r1, r2 = et("r1"), et("r2")
    nc.scalar.activation(out=r1[:], in_=l1[:], func=A.Exp, scale=-0.5)
    nc.scalar.activation(out=r2[:], in_=l2[:], func=A.Exp, scale=-0.5)
    pos, neg = et("pos"), et("neg")
    nc.vector.tensor_mul(pos[:], dxy1[:], r1[:])
    nc.vector.tensor_mul(neg[:], dxy2[:], r2[:])
    z = et("z")
    nc.vector.tensor_sub(z[:], neg[:], pos[:])
    e = et("e")
    nc.scalar.activation(out=e[:], in_=z[:], func=A.Exp)
    loss = et("loss")
    nc.scalar.activation(out=loss[:], in_=e[:], func=A.Ln, bias=1.0)
    nc.sync.dma_start(out=out_ap, in_=loss[:])


def _build():
    nc = bacc.Bacc("TRN2", target_bir_lowering=False, debug=False,
                   num_devices=N_CORES)
    x = nc.dram_tensor("x", [128, NBLK * D], DT_IN, kind="ExternalInput").ap()
    y1 = nc.dram_tensor("y1", [128, NBLK * D], DT_IN,
                        kind="ExternalInput").ap()
    y2 = nc.dram_tensor("y2", [128, NBLK * D], DT_IN,
                        kind="ExternalInput").ap()
    out = nc.dram_tensor("out", [128, NBLK], F32, kind="ExternalOutput").ap()
    with tile.TileContext(nc) as tc:
        with ExitStack() as ctx:
            _body(ctx, tc, out[:], x[:], y1[:], y2[:])
    nc.compile()
    return nc


_NC_CACHE = None


def _get_nc():
    global _NC_CACHE
    if _NC_CACHE is None:
        _NC_CACHE = _build()
    return _NC_CACHE


def _layout(a_rows):
    # [4096, 512] -> partition-major [128, 32*512]: partition p holds rows
    # {blk*128+p}, each row's 512 elems contiguous (8 KiB DMA lines at
    # SUP=8 in bf16).
    import ml_dtypes
    a = a_rows.reshape(NBLK, 128, D).transpose(1, 0, 2).reshape(128, NBLK * D)
    return np.ascontiguousarray(a.astype(ml_dtypes.bfloat16))


def _in_maps(query, embed):
    x1 = query[0::2]
    e1 = embed[0::2]
    e2 = embed[1::2]
    maps = []
    for c in range(N_CORES):
        sl = slice(c * ROWS_PER_CORE, (c + 1) * ROWS_PER_CORE)
        maps.append({"x": _layout(x1[sl]), "y1": _layout(e1[sl]),
                     "y2": _layout(e2[sl])})
    return maps


def kernel(query, embed, y, _trace=False):
    query = np.asarray(query, dtype=np.float32)
    embed = np.asarray(embed, dtype=np.float32)
    nc = _get_nc()
    res = run_bass_kernel_spmd(nc, _in_maps(query, embed),
                               core_ids=list(range(N_CORES)), trace=_trace)
    total = 0.0
    for c in range(N_CORES):
        total += res.results[c]["out"].astype(np.float64).sum()
    if _trace:
        kernel._last_results = res
    return np.float32(total / PAIRS)
